# revision 15
# baseline (speedup 1.0000x reference)
"""CausalBank kernel v6: bf16, N=512 W2 matmuls, replicated upstream (no h-AG),
quarter-pipelined W2 with AG-half-ordered k accumulation.

Per-core work:
  A) full embedding gather (bf16) + PE transpose -> featT emb part
  B) full u/a matmuls + 8 per-mode scans per batch (replicated; h-AllGather
     eliminated) -> featT h part
  C) full router -> gates -> this core's expert gate row (one-hot matmul)
  D) W1 for the core's 4 (e,h) k-tiles, gated -> per-quarter AllGather
     (quarter 0 split into two halves so W2 can start on half 1)
  E) W2 over the core's 4000-wide vocab shard: per quarter, 8 vocab chunks
     (7x512 + 416), 32-deep k accumulation at N=512. W2 k-axis is host-permuted
     to AllGather arrival order (s = j_local*8 + core).
"""

import os
import sys

for _p in ("/opt/trn_rl_repo",):
    if _p not in sys.path and os.path.isdir(_p):
        sys.path.insert(0, _p)

import numpy as np
import ml_dtypes

import concourse.bass as bass
import concourse.bacc as bacc
import concourse.mybir as mybir
import concourse.tile as tile
from concourse.bass import ts, ds
from concourse.bass_utils import run_bass_kernel_spmd
from concourse.masks import make_identity

B, S, D, M, H, E, V = 2, 1024, 512, 1024, 1024, 4, 32000
BS = B * S
F = M + D
NCORES = 8
VS = V // NCORES       # 4000
P = 128
DT = D // P            # 4
MT = M // P            # 8
FT = F // P            # 12
HT = H // P            # 8
KH = E * HT            # 32 k-tiles of the readout contraction
KC = KH // NCORES      # 4 k-tiles of W1 per core
QN = 4
QBS = BS // QN         # 512
VCN = 8                # vocab chunks per core: 7x512 + 1x416
VCW = [512] * 7 + [416]
VCO = [512 * i for i in range(VCN)]
NCB = S // 512         # 512-token chunks per batch
BF = mybir.dt.bfloat16
F32 = mybir.dt.float32
AF = mybir.ActivationFunctionType
OP = mybir.AluOpType

_CACHE = {}
LAST_EXEC_NS = None


def _install_ntff_hook():
    import contextlib
    import ctypes
    import types

    if "antenv.axon_hooks" in sys.modules:
        return
    so_path = "/opt/axon/libaxon_pjrt.so"
    hook = None
    if os.path.exists(so_path):
        lib = ctypes.CDLL(so_path)
        if hasattr(lib, "axon_start_nrt_profile"):
            lib.axon_start_nrt_profile.argtypes = [
                ctypes.POINTER(ctypes.c_int64),
                ctypes.c_size_t,
            ]
            lib.axon_start_nrt_profile.restype = ctypes.c_int64
            lib.axon_stop_nrt_profile.argtypes = [ctypes.c_char_p]
            lib.axon_stop_nrt_profile.restype = ctypes.c_int64

            @contextlib.contextmanager
            def hook(output_dir, device_ids):
                import jax

                jax.devices()
                if device_ids:
                    ids = (ctypes.c_int64 * len(device_ids))(*device_ids)
                    rc = lib.axon_start_nrt_profile(ids, len(device_ids))
                else:
                    rc = lib.axon_start_nrt_profile(None, 0)
                if rc != 0:
                    raise RuntimeError(f"axon_start_nrt_profile rc={rc}")
                try:
                    yield
                finally:
                    n = lib.axon_stop_nrt_profile(str(output_dir).encode())
                    if n < 0:
                        raise RuntimeError(f"axon_stop_nrt_profile rc={n}")

    mod = types.ModuleType("antenv.axon_hooks")
    mod.get_axon_ntff_profile_hook = lambda: hook
    mod.set_axon_ntff_profile_hook = lambda h: None
    import antenv

    antenv.axon_hooks = mod
    sys.modules["antenv.axon_hooks"] = mod


def build_program(with_b2=False):
    nc = bacc.Bacc("TRN2", target_bir_lowering=False, debug=False)
    ALL = [list(range(NCORES))]

    tokens = nc.dram_tensor("tokens", [BS // P, P, 1], mybir.dt.int32, kind="ExternalInput")
    embed = nc.dram_tensor("embed", [V, D], BF, kind="ExternalInput")
    inproj = nc.dram_tensor("inproj", [DT, P, M], BF, kind="ExternalInput")
    gatew = nc.dram_tensor("gatew", [DT, P, M], BF, kind="ExternalInput")
    gateb = nc.dram_tensor("gateb", [P, MT], F32, kind="ExternalInput")
    routerw = nc.dram_tensor("routerw", [FT, P, E], BF, kind="ExternalInput")
    routerb = nc.dram_tensor("routerb", [E, 1], F32, kind="ExternalInput")
    gsel = nc.dram_tensor("gsel", [E, 1], F32, kind="ExternalInput")
    # this core's 4 (e,h) blocks of W1: [j, f_partition, f_tile, h_col]
    w1 = nc.dram_tensor("w1", [KC, P, FT, P], BF, kind="ExternalInput")
    b1 = nc.dram_tensor("b1", [P, KC], F32, kind="ExternalInput")
    # k-permuted vocab-chunked W2 shard: [vc, h_partition, k_slot, vcol(512)]
    w2d = nc.dram_tensor("w2d", [VCN, P, KH, 512], BF, kind="ExternalInput")
    b2 = nc.dram_tensor("b2", [E, VS], BF, kind="ExternalInput")
    out = nc.dram_tensor("out", [BS, VS], F32, kind="ExternalOutput")

    with tile.TileContext(nc) as tc:
        with (
            tc.tile_pool(name="const", bufs=1) as const,
            tc.tile_pool(name="persist", bufs=1) as persist,
            tc.tile_pool(name="up", bufs=1) as up,
            tc.tile_pool(name="gath", bufs=3) as gath,
            tc.tile_pool(name="w2p", bufs=1) as w2p,
            tc.tile_pool(name="hidp", bufs=1) as hidp,
            tc.tile_pool(name="drainp", bufs=1) as drainp,
            tc.tile_pool(name="dram", bufs=1, space="DRAM") as dpool,
            tc.tile_pool(name="ps_ua", bufs=1, space="PSUM") as ps_ua,
            tc.tile_pool(name="ps_t", bufs=1, space="PSUM") as ps_t,
            tc.tile_pool(name="ps_r", bufs=1, space="PSUM") as ps_r,
            tc.tile_pool(name="ps_h", bufs=1, space="PSUM") as ps_h,
            tc.tile_pool(name="ps_o", bufs=1, space="PSUM") as ps_o,
        ):
            # ---------------- constants ----------------
            ident = const.tile([P, P], BF)
            make_identity(nc, ident[:])
            gateb_sb = const.tile([P, MT], F32)
            nc.sync.dma_start(gateb_sb[:], gateb[:])
            rw_sb = const.tile([P, FT, E], BF)
            nc.sync.dma_start(rw_sb[:], routerw[:].rearrange("f p e -> p f e"))
            rb_sb = const.tile([E, 1], F32)
            nc.sync.dma_start(rb_sb[:], routerb[:])
            ones44 = const.tile([E, E], F32)
            nc.any.memset(ones44[:], 1.0)
            b1_sb = const.tile([P, KC], F32)
            nc.sync.dma_start(b1_sb[:], b1[:])
            gsel_sb = const.tile([E, 1], F32)
            nc.sync.dma_start(gsel_sb[:], gsel[:])
            w1_sb = const.tile([P, KC, FT, P], BF)
            nc.sync.dma_start(w1_sb[:], w1[:].rearrange("j p f c -> p j f c"))
            if with_b2:
                b2_sb = const.tile([P, VS], BF)
                nc.any.memset(b2_sb[:], 0.0)
                nc.sync.dma_start(b2_sb[:E, :], b2[:])

            featT = persist.tile([P, FT, BS], BF)       # 6.3 MB
            gatesT = persist.tile([E, BS], F32)
            g_row = persist.tile([1, BS], F32)
            if with_b2:
                gb_sb = persist.tile([P, BS], BF)
                nc.any.memset(gb_sb[:], 0.0)
            gdram1 = dpool.tile([1, BS], F32)

            # PE warm-up: flip HAM early
            wm = up.tile([P, 512], BF)
            nc.any.memset(wm[:], 0.5)
            for w in range(12):
                wps = ps_h.tile([P, 512], F32, tag="psh", bufs=1)
                nc.tensor.matmul(wps[:], wm[:, 0:P], wm[:], start=True, stop=True)

            # ---------------- W2 tile prefetch (quarter 0, vc 0) ---------------
            w2_pre = {}
            w2a0 = w2p.tile([P, 16, 512], BF, tag="w2", bufs=3, name="w2a_p0")
            nc.sync.dma_start(w2a0[:], w2d[0][:, 0:16, :])
            w2b0 = w2p.tile([P, 16, 512], BF, tag="w2", bufs=3, name="w2b_p0")
            nc.sync.dma_start(w2b0[:], w2d[0][:, 16:32, :])
            w2_pre[(0, 0)] = (w2a0, w2b0)

            # ---------------- upstream helpers ----------------
            def emit_gathers(b):
                for i in range(BS // P // B):
                    ii = b * 8 + i
                    tok_t = gath.tile([P, 1], mybir.dt.int32, tag="tok", bufs=2)
                    nc.sync.dma_start(tok_t[:], tokens[ii])
                    emb_t = gath.tile([P, D], BF, tag="emb", bufs=2)
                    nc.gpsimd.indirect_dma_start(
                        out=emb_t[:], out_offset=None, in_=embed[:],
                        in_offset=bass.IndirectOffsetOnAxis(ap=tok_t[:, :1], axis=0),
                    )
                    for d in range(DT):
                        pst = ps_t.tile([P, P], BF, tag="pst", bufs=1)
                        nc.tensor.transpose(pst[:], emb_t[:, ts(d, P)], ident[:])
                        nc.vector.tensor_copy(featT[:, MT + d, ts(ii, P)], pst[:])

            def emit_ua_scan(b):
                bsl = ts(b, S)
                for m in range(MT):
                    ip_m = up.tile([P, DT, P], BF, tag="ipm", bufs=2, name=f"ip{b}_{m}")
                    nc.sync.dma_start(
                        ip_m[:], inproj[:, :, ts(m, P)].rearrange("d p m -> p d m")
                    )
                    gw_m = up.tile([P, DT, P], BF, tag="gwm", bufs=2, name=f"gw{b}_{m}")
                    nc.sync.dma_start(
                        gw_m[:], gatew[:, :, ts(m, P)].rearrange("d p m -> p d m")
                    )
                    for c in range(NCB):
                        cc = b * NCB + c
                        psu = ps_ua.tile([P, 512], F32, tag="psu", bufs=2)
                        for d in range(DT):
                            nc.tensor.matmul(
                                psu[:], ip_m[:, d, :],
                                featT[:, MT + d, ts(cc, 512)],
                                start=(d == 0), stop=(d == DT - 1),
                            )
                        psa = ps_ua.tile([P, 512], F32, tag="psa", bufs=1)
                        for d in range(DT):
                            nc.tensor.matmul(
                                psa[:], gw_m[:, d, :],
                                featT[:, MT + d, ts(cc, 512)],
                                start=(d == 0), stop=(d == DT - 1),
                            )
                        a_c = up.tile([P, 512], F32, tag="a", bufs=2, name=f"a{b}_{m}_{c}")
                        nc.scalar.activation(
                            a_c[:], psa[:], AF.Sigmoid,
                            bias=gateb_sb[:, m : m + 1], scale=1.0,
                        )
                        # chunked scan: fp32 state, bf16 carry across chunks
                        init = 0.0 if c == 0 else featT[:, m, cc * 512 - 1 : cc * 512]
                        nc.vector.tensor_tensor_scan(
                            out=featT[:, m, ts(cc, 512)], data0=a_c[:], data1=psu[:],
                            initial=init, op0=OP.mult, op1=OP.add,
                        )

            def emit_router(b):
                bsl = ts(b, S)
                gexp = up.tile([E, S], F32, tag="gexp", bufs=2, name=f"gexp{b}")
                rsum4 = up.tile([E, S], F32, tag="rsum", bufs=2, name=f"rsum{b}")
                for c in range(NCB):
                    cc = b * NCB + c
                    psr = ps_r.tile([E, 512], F32, tag="psr", bufs=1)
                    for f in range(FT):
                        nc.tensor.matmul(
                            psr[:], rw_sb[:, f, :], featT[:, f, ts(cc, 512)],
                            start=(f == 0), stop=(f == FT - 1),
                        )
                    nc.scalar.activation(
                        gexp[:, ts(c, 512)], psr[:], AF.Exp, bias=rb_sb[:], scale=1.0
                    )
                for c in range(NCB):
                    pss = ps_r.tile([E, 512], F32, tag="psr", bufs=1)
                    nc.tensor.matmul(
                        pss[:], ones44[:], gexp[:, ts(c, 512)], start=True, stop=True
                    )
                    nc.vector.reciprocal(rsum4[:, ts(c, 512)], pss[:])
                nc.vector.tensor_tensor(
                    out=gatesT[:, bsl], in0=gexp[:], in1=rsum4[:], op=OP.mult
                )
                if with_b2:
                    nc.vector.tensor_copy(gb_sb[:E, bsl], gatesT[:, bsl])
                for c in range(NCB):
                    cc = b * NCB + c
                    psg = ps_r.tile([E, 512], F32, tag="psr", bufs=1)
                    nc.tensor.matmul(
                        psg[0:1, :], gsel_sb[:], gatesT[:, ts(cc, 512)],
                        start=True, stop=True,
                    )
                    nc.vector.tensor_copy(g_row[:, ts(cc, 512)], psg[0:1, :])
                nc.sync.dma_start(gdram1[:, bsl], g_row[:, bsl])

            # W1 for one quarter; quarter 0 AG is split into two k-halves
            hid_parts = []     # per quarter: list of (dram_out, jj0, njj)
            def emit_w1(q):
                qsl = ds(q * QBS, QBS)
                g_t = up.tile([P, QBS], F32, tag="gt", bufs=2, name=f"gt{q}")
                nc.sync.dma_start(g_t[:], gdram1[0:1, qsl].to_broadcast((P, QBS)))
                split = q == 0
                if split:
                    hins = [
                        dpool.tile([P, 2, QBS], BF, name=f"hinA{q}"),
                        dpool.tile([P, 2, QBS], BF, name=f"hinB{q}"),
                    ]
                    houts = [
                        dpool.tile([NCORES, P, 2, QBS], BF, addr_space="Shared", name=f"houtA{q}"),
                        dpool.tile([NCORES, P, 2, QBS], BF, addr_space="Shared", name=f"houtB{q}"),
                    ]
                    parts = [(houts[0], 0, 2), (houts[1], 2, 2)]
                else:
                    hins = [dpool.tile([P, KC, QBS], BF, name=f"hin{q}")]
                    houts = [dpool.tile([NCORES, P, KC, QBS], BF, addr_space="Shared", name=f"hout{q}")]
                    parts = [(houts[0], 0, 4)]
                for j in range(KC):
                    psh = ps_h.tile([P, 512], F32, tag="psh", bufs=1)
                    for f in range(FT):
                        nc.tensor.matmul(
                            psh[:], w1_sb[:, j, f, :], featT[:, f, qsl],
                            start=(f == 0), stop=(f == FT - 1),
                        )
                    r_t = up.tile([P, QBS], F32, tag="relu", bufs=2, name=f"r{q}_{j}")
                    nc.scalar.activation(
                        r_t[:], psh[:], AF.Relu, bias=b1_sb[:, j : j + 1], scale=1.0
                    )
                    nc.vector.tensor_tensor(out=r_t[:], in0=r_t[:], in1=r_t[:], op=OP.mult)
                    hl_t = up.tile([P, QBS], BF, tag="hl", bufs=2, name=f"hl{q}_{j}")
                    nc.vector.tensor_tensor(out=hl_t[:], in0=r_t[:], in1=g_t[:], op=OP.mult)
                    if split:
                        nc.sync.dma_start(hins[j // 2][:, j % 2, :], hl_t[:])
                        if j % 2 == 1:
                            nc.gpsimd.collective_compute(
                                "AllGather", OP.bypass, replica_groups=ALL,
                                ins=[hins[j // 2][:]], outs=[houts[j // 2][:]],
                            )
                    else:
                        nc.sync.dma_start(hins[0][:, j, :], hl_t[:])
                        if j == KC - 1:
                            nc.gpsimd.collective_compute(
                                "AllGather", OP.bypass, replica_groups=ALL,
                                ins=[hins[0][:]], outs=[houts[0][:]],
                            )
                hid_parts.append(parts)

            # assemble hidT for quarter q: [P, KC(jj), NCORES(r), QBS], slot s = jj*8+r
            def assemble_hidT(q):
                hidT = hidp.tile([P, KC, NCORES, QBS], BF, tag="hidT", bufs=1, name=f"hidT{q}")
                for hout, jj0, njj in hid_parts[q]:
                    for r in range(NCORES):
                        nc.sync.dma_start(hidT[:, ds(jj0, njj), r, :], hout[r])
                return hidT

            def emit_w2(q, hidT):
                for vc in range(VCN):
                    cw = VCW[vc]
                    voff = VCO[vc]
                    if (q, vc) in w2_pre:
                        w2a, w2b = w2_pre[(q, vc)]
                    else:
                        w2a = w2p.tile([P, 16, 512], BF, tag="w2", bufs=3, name=f"w2a{q}_{vc}")
                        nc.sync.dma_start(w2a[:], w2d[vc][:, 0:16, :])
                        w2b = w2p.tile([P, 16, 512], BF, tag="w2", bufs=3, name=f"w2b{q}_{vc}")
                        nc.sync.dma_start(w2b[:], w2d[vc][:, 16:32, :])
                    for bt in range(QBS // P):
                        row0 = q * QBS + bt * P
                        pso = ps_o.tile([P, 512], F32, tag="pso", bufs=2)
                        for s in range(KH):
                            wt = w2a if s < 16 else w2b
                            nc.tensor.matmul(
                                pso[:, 0:cw],
                                hidT[:, s // 8, s % 8, ts(bt, P)],
                                wt[:, s % 16, 0:cw],
                                start=(s == 0),
                                stop=(s == KH - 1 and not with_b2),
                            )
                        if with_b2:
                            nc.tensor.matmul(
                                pso[:, 0:cw], gb_sb[:, ds(row0, P)],
                                b2_sb[:, ds(voff, cw)], start=False, stop=True,
                            )
                        o_t = drainp.tile([P, 512], F32, tag="ot", bufs=2, name=f"ot{q}_{vc}_{bt}")
                        nc.vector.tensor_copy(o_t[:, 0:cw], pso[:, 0:cw])
                        nc.sync.dma_start(out[ds(row0, P), ds(voff, cw)], o_t[:, 0:cw])

            # ---------------- emission order (= scheduler priority) -----------
            emit_gathers(0)
            emit_ua_scan(0)
            emit_gathers(1)
            emit_router(0)
            emit_w1(0)
            emit_w1(1)
            hidT0 = assemble_hidT(0)
            hidT1 = assemble_hidT(1)
            emit_ua_scan(1)
            emit_router(1)
            emit_w1(2)
            emit_w1(3)
            hidT2 = assemble_hidT(2)
            hidT3 = assemble_hidT(3)
            emit_w2(0, hidT0)
            emit_w2(1, hidT1)
            emit_w2(2, hidT2)
            emit_w2(3, hidT3)

    nc.compile()
    return nc


def _to_bf16(x):
    return np.asarray(x, dtype=np.float32).astype(ml_dtypes.bfloat16)


def prepare_in_maps(inputs, ncores=NCORES):
    tokens = np.asarray(inputs["tokens"]).astype(np.int32).reshape(BS // P, P, 1)
    embed = np.ascontiguousarray(_to_bf16(inputs["embed"]))
    inproj = np.ascontiguousarray(_to_bf16(inputs["in_proj"]).reshape(DT, P, M))
    gatew = np.ascontiguousarray(_to_bf16(inputs["gate_w"]).reshape(DT, P, M))
    gateb = np.ascontiguousarray(
        np.asarray(inputs["gate_b"], dtype=np.float32).reshape(MT, P).T
    )
    routerw_bf = _to_bf16(inputs["router_w"]).reshape(FT, P, E)
    routerb = np.asarray(inputs["router_b"], dtype=np.float32).reshape(E, 1)
    w1_bf = _to_bf16(inputs["w1"]).reshape(E, FT, P, HT, P).transpose(0, 3, 2, 1, 4)
    w1_k = np.ascontiguousarray(w1_bf.reshape(KH, P, FT, P))
    b1_k = np.asarray(inputs["b1"], dtype=np.float32).reshape(E, HT, P).reshape(KH, P)
    w2_f = np.asarray(inputs["w2"], dtype=np.float32).reshape(KH, P, V)
    b2_bf = _to_bf16(inputs["b2"])
    # k-slot permutation: slot s = jj*8 + r  ->  global k = r*KC + jj
    perm = np.array([(s % 8) * KC + (s // 8) for s in range(KH)], dtype=np.int64)

    shared = dict(
        tokens=tokens, embed=embed, inproj=inproj, gatew=gatew, gateb=gateb,
        routerw=np.ascontiguousarray(routerw_bf), routerb=routerb,
    )
    in_maps = []
    for c in range(ncores):
        m = dict(shared)
        onehot = np.zeros((E, 1), np.float32)
        onehot[c // 2, 0] = 1.0
        m["gsel"] = onehot
        m["w1"] = np.ascontiguousarray(w1_k[c * KC : (c + 1) * KC])
        m["b1"] = np.ascontiguousarray(b1_k[c * KC : (c + 1) * KC].T)
        w2c = w2_f[perm][:, :, c * VS : (c + 1) * VS]          # [KH, P, VS] f32
        w2pad = np.zeros((KH, P, VCN * 512), np.float32)
        w2pad[:, :, :VS] = w2c
        m["w2d"] = np.ascontiguousarray(
            _to_bf16(w2pad.reshape(KH, P, VCN, 512).transpose(2, 1, 0, 3))
        )
        m["b2"] = np.ascontiguousarray(b2_bf[:, c * VS : (c + 1) * VS])
        in_maps.append(m)
    return in_maps


def kernel(**inputs):
    global LAST_EXEC_NS
    trace = os.environ.get("BASS_TRACE", "") not in ("", "0")
    if trace:
        _install_ntff_hook()
    with_b2 = bool(np.any(np.asarray(inputs["b2"])))
    key = ("nc", with_b2)
    if key not in _CACHE:
        _CACHE[key] = build_program(with_b2=with_b2)
    nc = _CACHE[key]
    in_maps = prepare_in_maps(inputs)
    res = run_bass_kernel_spmd(nc, in_maps, list(range(NCORES)), trace=trace)
    LAST_EXEC_NS = res.exec_time_ns
    parts = [res.results[c]["out"] for c in range(NCORES)]
    full = np.concatenate(parts, axis=1).reshape(B, S, V).astype(np.float32)
    return full


# revision 20
# speedup vs baseline: 1.0038x; 1.0038x over previous
"""CausalBank kernel v7: bf16 W2 at rate-floor, latency-optimized head.

Per-core work:
  A) full embedding gather (bf16) + PE transpose -> featT emb part
  B) u/a matmuls + chunked scan (PSUM-direct, bf16 carry) for the core's
     128-mode tile; h AllGather (batch 0 split into S-halves so router/W1 of
     quarter 0 start after the first half lands)
  C) full router (fast-NR reciprocal) -> this core's expert gate row
  D) W1 for the core's 4 (e,h) k-tiles, gated -> per-quarter AllGather
     (quarter 0 split into two k-halves so W2 starts on half 1)
  E) W2 over the core's 4000-wide vocab shard: per quarter, 8 vocab chunks
     (7x512 + 416), 32-deep k accumulation at N=512. W2 k-axis host-permuted
     to AllGather arrival order (slot s = j_local*8 + core).
"""

import os
import sys

for _p in ("/opt/trn_rl_repo",):
    if _p not in sys.path and os.path.isdir(_p):
        sys.path.insert(0, _p)

import numpy as np
import ml_dtypes

import concourse.bass as bass
import concourse.bacc as bacc
import concourse.mybir as mybir
import concourse.tile as tile
from concourse.bass import ts, ds
from concourse.bass_utils import run_bass_kernel_spmd
from concourse.masks import make_identity

B, S, D, M, H, E, V = 2, 1024, 512, 1024, 1024, 4, 32000
BS = B * S
F = M + D
NCORES = 8
VS = V // NCORES       # 4000
P = 128
DT = D // P            # 4
MT = M // P            # 8
FT = F // P            # 12
HT = H // P            # 8
KH = E * HT            # 32 k-tiles of the readout contraction
KC = KH // NCORES      # 4 k-tiles of W1 per core
QN = 4
QBS = BS // QN         # 512
VCN = 8                # vocab chunks per core: 7x512 + 1x416
VCW = [512] * 7 + [416]
VCO = [512 * i for i in range(VCN)]
NCB = S // 512         # 512-token chunks per batch
BF = mybir.dt.bfloat16
F32 = mybir.dt.float32
AF = mybir.ActivationFunctionType
OP = mybir.AluOpType

_CACHE = {}
LAST_EXEC_NS = None


def _install_ntff_hook():
    import contextlib
    import ctypes
    import types

    if "antenv.axon_hooks" in sys.modules:
        return
    so_path = "/opt/axon/libaxon_pjrt.so"
    hook = None
    if os.path.exists(so_path):
        lib = ctypes.CDLL(so_path)
        if hasattr(lib, "axon_start_nrt_profile"):
            lib.axon_start_nrt_profile.argtypes = [
                ctypes.POINTER(ctypes.c_int64),
                ctypes.c_size_t,
            ]
            lib.axon_start_nrt_profile.restype = ctypes.c_int64
            lib.axon_stop_nrt_profile.argtypes = [ctypes.c_char_p]
            lib.axon_stop_nrt_profile.restype = ctypes.c_int64

            @contextlib.contextmanager
            def hook(output_dir, device_ids):
                import jax

                jax.devices()
                if device_ids:
                    ids = (ctypes.c_int64 * len(device_ids))(*device_ids)
                    rc = lib.axon_start_nrt_profile(ids, len(device_ids))
                else:
                    rc = lib.axon_start_nrt_profile(None, 0)
                if rc != 0:
                    raise RuntimeError(f"axon_start_nrt_profile rc={rc}")
                try:
                    yield
                finally:
                    n = lib.axon_stop_nrt_profile(str(output_dir).encode())
                    if n < 0:
                        raise RuntimeError(f"axon_stop_nrt_profile rc={n}")

    mod = types.ModuleType("antenv.axon_hooks")
    mod.get_axon_ntff_profile_hook = lambda: hook
    mod.set_axon_ntff_profile_hook = lambda h: None
    import antenv

    antenv.axon_hooks = mod
    sys.modules["antenv.axon_hooks"] = mod


def build_program(with_b2=False):
    nc = bacc.Bacc("TRN2", target_bir_lowering=False, debug=False)
    ALL = [list(range(NCORES))]

    tokens = nc.dram_tensor("tokens", [BS // P, P, 1], mybir.dt.int32, kind="ExternalInput")
    embed = nc.dram_tensor("embed", [V, D], BF, kind="ExternalInput")
    # per-core column slice of in_proj / gate_w (this core's mode tile)
    inproj = nc.dram_tensor("inproj", [DT, P, P], BF, kind="ExternalInput")
    gatew = nc.dram_tensor("gatew", [DT, P, P], BF, kind="ExternalInput")
    gateb = nc.dram_tensor("gateb", [P, 1], F32, kind="ExternalInput")
    routerw = nc.dram_tensor("routerw", [FT, P, E], BF, kind="ExternalInput")
    routerb = nc.dram_tensor("routerb", [E, 1], F32, kind="ExternalInput")
    gsel = nc.dram_tensor("gsel", [E, 1], F32, kind="ExternalInput")
    # this core's 4 (e,h) blocks of W1: [j, f_partition, f_tile, h_col]
    w1 = nc.dram_tensor("w1", [KC, P, FT, P], BF, kind="ExternalInput")
    b1 = nc.dram_tensor("b1", [P, KC], F32, kind="ExternalInput")
    # k-permuted vocab-chunked W2 shard: [vc, h_partition, k_slot, vcol(512)]
    w2d = nc.dram_tensor("w2d", [VCN, P, KH, 512], BF, kind="ExternalInput")
    b2 = nc.dram_tensor("b2", [E, VS], BF, kind="ExternalInput")
    out = nc.dram_tensor("out", [BS, VS], F32, kind="ExternalOutput")

    with tile.TileContext(nc) as tc:
        with (
            tc.tile_pool(name="const", bufs=1) as const,
            tc.tile_pool(name="persist", bufs=1) as persist,
            tc.tile_pool(name="up", bufs=1) as up,
            tc.tile_pool(name="gath", bufs=2) as gath,
            tc.tile_pool(name="w2p", bufs=1) as w2p,
            tc.tile_pool(name="hidp", bufs=1) as hidp,
            tc.tile_pool(name="drainp", bufs=1) as drainp,
            tc.tile_pool(name="dram", bufs=1, space="DRAM") as dpool,
            tc.tile_pool(name="ps_ua", bufs=1, space="PSUM") as ps_ua,
            tc.tile_pool(name="ps_t", bufs=1, space="PSUM") as ps_t,
            tc.tile_pool(name="ps_r", bufs=1, space="PSUM") as ps_r,
            tc.tile_pool(name="ps_h", bufs=1, space="PSUM") as ps_h,
            tc.tile_pool(name="ps_o", bufs=1, space="PSUM") as ps_o,
        ):
            # identity first: it's on the gpsimd queue ahead of the gathers
            ident = const.tile([P, P], BF)
            make_identity(nc, ident[:])

            featT = persist.tile([P, FT, BS], BF)       # 6.3 MB
            gatesT = persist.tile([E, BS], F32)
            g_row = persist.tile([1, BS], F32)
            if with_b2:
                gb_sb = persist.tile([P, BS], BF)
                nc.any.memset(gb_sb[:], 0.0)
            gdram1 = dpool.tile([1, BS], F32)

            # token indices via the Activation DMA queue (ahead of bulk loads)
            tok_ts = []
            for ii in range(BS // P):
                tok_t = gath.tile([P, 1], mybir.dt.int32, tag="tok", bufs=16)
                nc.scalar.dma_start(tok_t[:], tokens[ii])
                tok_ts.append(tok_t)

            def emit_gathers(b):
                for i in range(BS // P // B):
                    ii = b * 8 + i
                    emb_t = gath.tile([P, D], BF, tag="emb", bufs=2)
                    nc.gpsimd.indirect_dma_start(
                        out=emb_t[:], out_offset=None, in_=embed[:],
                        in_offset=bass.IndirectOffsetOnAxis(ap=tok_ts[ii][:, :1], axis=0),
                    )
                    for d in range(DT):
                        pst = ps_t.tile([P, P], BF, tag="pst", bufs=1)
                        nc.tensor.transpose(pst[:], emb_t[:, ts(d, P)], ident[:])
                        nc.vector.tensor_copy(featT[:, MT + d, ts(ii, P)], pst[:])

            emit_gathers(0)

            # ---------------- constants (bulk queue) ----------------
            gateb_sb = const.tile([P, 1], F32)
            nc.sync.dma_start(gateb_sb[:], gateb[:])
            rw_sb = const.tile([P, FT, E], BF)
            nc.sync.dma_start(rw_sb[:], routerw[:].rearrange("f p e -> p f e"))
            rb_sb = const.tile([E, 1], F32)
            nc.sync.dma_start(rb_sb[:], routerb[:])
            ones44 = const.tile([E, E], F32)
            nc.any.memset(ones44[:], 1.0)
            b1_sb = const.tile([P, KC], F32)
            nc.sync.dma_start(b1_sb[:], b1[:])
            gsel_sb = const.tile([E, 1], F32)
            nc.sync.dma_start(gsel_sb[:], gsel[:])
            inproj_sb = const.tile([P, DT, P], BF)
            nc.sync.dma_start(inproj_sb[:], inproj[:].rearrange("d p m -> p d m"))
            gatew_sb = const.tile([P, DT, P], BF)
            nc.sync.dma_start(gatew_sb[:], gatew[:].rearrange("d p m -> p d m"))
            w1_sb = const.tile([P, KC, FT, P], BF)
            nc.sync.dma_start(w1_sb[:], w1[:].rearrange("j p f c -> p j f c"))
            if with_b2:
                b2_sb = const.tile([P, VS], BF)
                nc.any.memset(b2_sb[:], 0.0)
                nc.sync.dma_start(b2_sb[:E, :], b2[:])

            # ---------------- W2 tile prefetch (quarter 0, vc 0) ---------------
            w2_pre = {}
            w2a0 = w2p.tile([P, 16, 512], BF, tag="w2", bufs=3, name="w2a_p0")
            nc.sync.dma_start(w2a0[:], w2d[0][:, 0:16, :])
            w2b0 = w2p.tile([P, 16, 512], BF, tag="w2", bufs=3, name="w2b_p0")
            nc.sync.dma_start(w2b0[:], w2d[0][:, 16:32, :])
            w2_pre[(0, 0)] = (w2a0, w2b0)

            # PE warm-up
            wm = up.tile([P, 512], BF)
            nc.any.memset(wm[:], 0.5)
            for w in range(8):
                wps = ps_h.tile([P, 512], F32, tag="psh", bufs=1)
                nc.tensor.matmul(wps[:], wm[:, 0:P], wm[:], start=True, stop=True)

            # ---------------- u/a + chunked scan (core's mode tile) ------------
            h_ins = [dpool.tile([P, S], BF, name=f"h_in{b}") for b in range(B)]
            h_outs = [
                dpool.tile([NCORES, P, S], BF, addr_space="Shared", name=f"h_out{b}")
                for b in range(B)
            ]
            # split-AG variants (batch 0): one contiguous in/Shared out per S-half
            h_ins0h = [
                dpool.tile([P, 512], BF, name=f"h_in0h{c}") for c in range(NCB)
            ]
            h_outs0h = [
                dpool.tile([NCORES, P, 512], BF, addr_space="Shared", name=f"h_out0h{c}")
                for c in range(NCB)
            ]

            def emit_ua_scan(b, split_ag):
                hT = up.tile([P, S], BF, tag="hT", bufs=2, name=f"hT{b}")
                for c in range(NCB):
                    cc = b * NCB + c
                    psu = ps_ua.tile([P, 512], F32, tag="psu", bufs=2)
                    for d in range(DT):
                        nc.tensor.matmul(
                            psu[:], inproj_sb[:, d, :], featT[:, MT + d, ts(cc, 512)],
                            start=(d == 0), stop=(d == DT - 1),
                        )
                    psa = ps_ua.tile([P, 512], F32, tag="psa", bufs=1)
                    for d in range(DT):
                        nc.tensor.matmul(
                            psa[:], gatew_sb[:, d, :], featT[:, MT + d, ts(cc, 512)],
                            start=(d == 0), stop=(d == DT - 1),
                        )
                    a_c = up.tile([P, 512], F32, tag="a", bufs=2, name=f"a{b}_{c}")
                    nc.scalar.activation(
                        a_c[:], psa[:], AF.Sigmoid, bias=gateb_sb[:, 0:1], scale=1.0
                    )
                    # chunked scan: fp32 state, PSUM-direct u, bf16 carry
                    init = 0.0 if c == 0 else hT[:, c * 512 - 1 : c * 512]
                    nc.vector.tensor_tensor_scan(
                        out=hT[:, ts(c, 512)], data0=a_c[:], data1=psu[:],
                        initial=init, op0=OP.mult, op1=OP.add,
                    )
                    if split_ag:
                        nc.scalar.dma_start(h_ins0h[c][:], hT[:, ts(c, 512)])
                        nc.gpsimd.collective_compute(
                            "AllGather", OP.bypass, replica_groups=ALL,
                            ins=[h_ins0h[c][:]],
                            outs=[h_outs0h[c][:]],
                        )
                        # featT h-part for this chunk (all cores' modes)
                        nc.sync.dma_start(
                            featT[:, 0:MT, ts(b * NCB + c, 512)],
                            h_outs0h[c][:].rearrange("r p s -> p r s"),
                        )
                if not split_ag:
                    nc.scalar.dma_start(h_ins[b][:], hT[:])
                    nc.gpsimd.collective_compute(
                        "AllGather", OP.bypass, replica_groups=ALL,
                        ins=[h_ins[b][:]], outs=[h_outs[b][:]],
                    )
                    nc.sync.dma_start(
                        featT[:, 0:MT, ts(b, S)],
                        h_outs[b][:].rearrange("r p s -> p r s"),
                    )

            # ---------------- router (per 512-chunk) ----------------
            def emit_router_chunk(b, c):
                cc = b * NCB + c
                gexp = up.tile([E, 512], F32, tag="gexp", bufs=2, name=f"gexp{b}_{c}")
                rs = up.tile([E, 512], F32, tag="rsum", bufs=2, name=f"rs{b}_{c}")
                scr = up.tile([E, 512], F32, tag="rscr", bufs=2, name=f"scr{b}_{c}")
                psr = ps_r.tile([E, 512], F32, tag="psr", bufs=1)
                for f in range(FT):
                    nc.tensor.matmul(
                        psr[:], rw_sb[:, f, :], featT[:, f, ts(cc, 512)],
                        start=(f == 0), stop=(f == FT - 1),
                    )
                nc.scalar.activation(
                    gexp[:], psr[:], AF.Exp, bias=rb_sb[:], scale=1.0
                )
                pss = ps_r.tile([E, 512], F32, tag="psr", bufs=1)
                nc.tensor.matmul(pss[:], ones44[:], gexp[:], start=True, stop=True)
                nc.vector.reciprocal_approx_accurate(out=rs[:], in_=pss[:], scratch=scr[:])
                nc.vector.tensor_tensor(
                    out=gatesT[:, ts(cc, 512)], in0=gexp[:], in1=rs[:], op=OP.mult
                )
                if with_b2:
                    nc.vector.tensor_copy(gb_sb[:E, ts(cc, 512)], gatesT[:, ts(cc, 512)])
                psg = ps_r.tile([E, 512], F32, tag="psr", bufs=1)
                nc.tensor.matmul(
                    psg[0:1, :], gsel_sb[:], gatesT[:, ts(cc, 512)],
                    start=True, stop=True,
                )
                nc.vector.tensor_copy(g_row[:, ts(cc, 512)], psg[0:1, :])
                nc.scalar.dma_start(gdram1[:, ts(cc, 512)], g_row[:, ts(cc, 512)])

            # ---------------- W1 quarter (+hid AllGather) ----------------
            hid_parts = {}     # q -> list of (dram_out, jj0, njj)
            def emit_w1(q):
                qsl = ds(q * QBS, QBS)
                g_t = up.tile([P, QBS], F32, tag="gt", bufs=2, name=f"gt{q}")
                nc.scalar.dma_start(g_t[:], gdram1[0:1, qsl].to_broadcast((P, QBS)))
                split = q == 0
                if split:
                    hins = [
                        dpool.tile([P, 2, QBS], BF, name=f"hinA{q}"),
                        dpool.tile([P, 2, QBS], BF, name=f"hinB{q}"),
                    ]
                    houts = [
                        dpool.tile([NCORES, P, 2, QBS], BF, addr_space="Shared", name=f"houtA{q}"),
                        dpool.tile([NCORES, P, 2, QBS], BF, addr_space="Shared", name=f"houtB{q}"),
                    ]
                    parts = [(houts[0], 0, 2), (houts[1], 2, 2)]
                else:
                    hins = [dpool.tile([P, KC, QBS], BF, name=f"hin{q}")]
                    houts = [dpool.tile([NCORES, P, KC, QBS], BF, addr_space="Shared", name=f"hout{q}")]
                    parts = [(houts[0], 0, 4)]
                for j in range(KC):
                    psh = ps_h.tile([P, 512], F32, tag="psh", bufs=1)
                    for f in range(FT):
                        nc.tensor.matmul(
                            psh[:], w1_sb[:, j, f, :], featT[:, f, qsl],
                            start=(f == 0), stop=(f == FT - 1),
                        )
                    r_t = up.tile([P, QBS], F32, tag="relu", bufs=2, name=f"r{q}_{j}")
                    nc.scalar.activation(
                        r_t[:], psh[:], AF.Relu, bias=b1_sb[:, j : j + 1], scale=1.0
                    )
                    nc.vector.tensor_tensor(out=r_t[:], in0=r_t[:], in1=r_t[:], op=OP.mult)
                    hl_t = up.tile([P, QBS], BF, tag="hl", bufs=2, name=f"hl{q}_{j}")
                    nc.vector.tensor_tensor(out=hl_t[:], in0=r_t[:], in1=g_t[:], op=OP.mult)
                    if split:
                        nc.scalar.dma_start(hins[j // 2][:, j % 2, :], hl_t[:])
                        if j % 2 == 1:
                            nc.gpsimd.collective_compute(
                                "AllGather", OP.bypass, replica_groups=ALL,
                                ins=[hins[j // 2][:]], outs=[houts[j // 2][:]],
                            )
                    else:
                        nc.scalar.dma_start(hins[0][:, j, :], hl_t[:])
                        if j == KC - 1:
                            nc.gpsimd.collective_compute(
                                "AllGather", OP.bypass, replica_groups=ALL,
                                ins=[hins[0][:]], outs=[houts[0][:]],
                            )
                hid_parts[q] = parts

            # hidT for quarter q: [P, KC(jj), NCORES(r), QBS], slot s = jj*8+r
            def assemble_hidT(q):
                hidT = hidp.tile([P, KC, NCORES, QBS], BF, tag="hidT", bufs=1, name=f"hidT{q}")
                for hout, jj0, njj in hid_parts[q]:
                    for r in range(NCORES):
                        nc.scalar.dma_start(hidT[:, ds(jj0, njj), r, :], hout[r])
                return hidT

            def emit_w2(q, hidT):
                for vc in range(VCN):
                    cw = VCW[vc]
                    voff = VCO[vc]
                    if (q, vc) in w2_pre:
                        w2a, w2b = w2_pre[(q, vc)]
                    else:
                        w2a = w2p.tile([P, 16, 512], BF, tag="w2", bufs=3, name=f"w2a{q}_{vc}")
                        nc.sync.dma_start(w2a[:], w2d[vc][:, 0:16, :])
                        w2b = w2p.tile([P, 16, 512], BF, tag="w2", bufs=3, name=f"w2b{q}_{vc}")
                        nc.sync.dma_start(w2b[:], w2d[vc][:, 16:32, :])
                    for bt in range(QBS // P):
                        row0 = q * QBS + bt * P
                        pso = ps_o.tile([P, 512], F32, tag="pso", bufs=2)
                        for s in range(KH):
                            wt = w2a if s < 16 else w2b
                            nc.tensor.matmul(
                                pso[:, 0:cw],
                                hidT[:, s // 8, s % 8, ts(bt, P)],
                                wt[:, s % 16, 0:cw],
                                start=(s == 0),
                                stop=(s == KH - 1 and not with_b2),
                            )
                        if with_b2:
                            nc.tensor.matmul(
                                pso[:, 0:cw], gb_sb[:, ds(row0, P)],
                                b2_sb[:, ds(voff, cw)], start=False, stop=True,
                            )
                        o_t = drainp.tile([P, 512], F32, tag="ot", bufs=2, name=f"ot{q}_{vc}_{bt}")
                        nc.vector.tensor_copy(o_t[:, 0:cw], pso[:, 0:cw])
                        nc.sync.dma_start(out[ds(row0, P), ds(voff, cw)], o_t[:, 0:cw])

            # ---------------- emission order (= scheduler priority) -----------
            emit_ua_scan(0, split_ag=True)
            emit_router_chunk(0, 0)
            emit_gathers(1)
            emit_router_chunk(0, 1)
            emit_w1(0)
            emit_w1(1)
            hidT0 = assemble_hidT(0)
            emit_ua_scan(1, split_ag=False)
            emit_router_chunk(1, 0)
            emit_router_chunk(1, 1)
            emit_w1(2)
            emit_w1(3)
            hidT1 = assemble_hidT(1)
            emit_w2(0, hidT0)
            hidT2 = assemble_hidT(2)
            emit_w2(1, hidT1)
            hidT3 = assemble_hidT(3)
            emit_w2(2, hidT2)
            emit_w2(3, hidT3)

    nc.compile()
    return nc


def _to_bf16(x):
    return np.asarray(x, dtype=np.float32).astype(ml_dtypes.bfloat16)


def prepare_in_maps(inputs, ncores=NCORES):
    tokens = np.asarray(inputs["tokens"]).astype(np.int32).reshape(BS // P, P, 1)
    embed = np.ascontiguousarray(_to_bf16(inputs["embed"]))
    inproj_f = np.asarray(inputs["in_proj"], dtype=np.float32)
    gatew_f = np.asarray(inputs["gate_w"], dtype=np.float32)
    gateb_f = np.asarray(inputs["gate_b"], dtype=np.float32)
    routerw_bf = np.ascontiguousarray(_to_bf16(inputs["router_w"]).reshape(FT, P, E))
    routerb = np.asarray(inputs["router_b"], dtype=np.float32).reshape(E, 1)
    w1_bf = _to_bf16(inputs["w1"]).reshape(E, FT, P, HT, P).transpose(0, 3, 2, 1, 4)
    w1_k = np.ascontiguousarray(w1_bf.reshape(KH, P, FT, P))
    b1_k = np.asarray(inputs["b1"], dtype=np.float32).reshape(E, HT, P).reshape(KH, P)
    w2_f = np.asarray(inputs["w2"], dtype=np.float32).reshape(KH, P, V)
    b2_bf = _to_bf16(inputs["b2"])
    # k-slot permutation: slot s = jj*8 + r  ->  global k = r*KC + jj
    perm = np.array([(s % 8) * KC + (s // 8) for s in range(KH)], dtype=np.int64)
    w2_p = w2_f[perm]

    shared = dict(tokens=tokens, embed=embed, routerb=routerb, routerw=routerw_bf)
    in_maps = []
    for c in range(ncores):
        m = dict(shared)
        msl = slice(c * P, (c + 1) * P)
        m["inproj"] = np.ascontiguousarray(_to_bf16(inproj_f[:, msl]).reshape(DT, P, P))
        m["gatew"] = np.ascontiguousarray(_to_bf16(gatew_f[:, msl]).reshape(DT, P, P))
        m["gateb"] = np.ascontiguousarray(gateb_f[msl].reshape(P, 1))
        onehot = np.zeros((E, 1), np.float32)
        onehot[c // 2, 0] = 1.0
        m["gsel"] = onehot
        m["w1"] = np.ascontiguousarray(w1_k[c * KC : (c + 1) * KC])
        m["b1"] = np.ascontiguousarray(b1_k[c * KC : (c + 1) * KC].T)
        w2pad = np.zeros((KH, P, VCN * 512), np.float32)
        w2pad[:, :, :VS] = w2_p[:, :, c * VS : (c + 1) * VS]
        m["w2d"] = np.ascontiguousarray(
            _to_bf16(w2pad.reshape(KH, P, VCN, 512).transpose(2, 1, 0, 3))
        )
        m["b2"] = np.ascontiguousarray(b2_bf[:, c * VS : (c + 1) * VS])
        in_maps.append(m)
    return in_maps


def kernel(**inputs):
    global LAST_EXEC_NS
    trace = os.environ.get("BASS_TRACE", "") not in ("", "0")
    if trace:
        _install_ntff_hook()
    with_b2 = bool(np.any(np.asarray(inputs["b2"])))
    key = ("nc", with_b2)
    if key not in _CACHE:
        _CACHE[key] = build_program(with_b2=with_b2)
    nc = _CACHE[key]
    in_maps = prepare_in_maps(inputs)
    res = run_bass_kernel_spmd(nc, in_maps, list(range(NCORES)), trace=trace)
    LAST_EXEC_NS = res.exec_time_ns
    parts = [res.results[c]["out"] for c in range(NCORES)]
    full = np.concatenate(parts, axis=1).reshape(B, S, V).astype(np.float32)
    return full
